# revision 1
# baseline (speedup 1.0000x reference)
"""Trainium2 Bass kernel for dynamic-filter 4x upsampling (nn_G_61856118997290).

Math: fw = softmax(filt, axis=1) over 343 taps; per color channel c the
output is pixel-shuffle(sum_p patches(x_c)[p] * fw[p, u]) for u in 0..16.

Computed as exp(filt) streams: N_c = sum_p P_c*E, S = sum_p E, out = N_c/S
(softmax normalization folded into one final division on the host).

Sharding: output rows H=128 split 8 ways (16 rows/core). Per core:
 - E-stream: filt slab [2,343,16,16,128] f32 (90MB) -> ACT exp -> bf16
 - patches P (host im2col, bf16) -> DVE multiply -> Z = P*E
 - PE ones-stationary matmuls reduce the 343-tap partition axis into PSUM
   partition groups {0,32,64} (M=32 replicated), 3 chunks accumulated
 - ACT/DVE evacuate PSUM -> SBUF -> DMA to DRAM
 - host: divide by S, pixel-shuffle, concat cores.
"""
import numpy as np
import ml_dtypes

import concourse.bass as bass
import concourse.tile as tile
from concourse import bacc, mybir
from concourse.bass_utils import run_bass_kernel_spmd

F32 = mybir.dt.float32
BF16 = mybir.dt.bfloat16
EXP = mybir.ActivationFunctionType.Exp

B, C, T, H, W = 2, 3, 7, 128, 128
NHB, PAD, UF = 7, 3, 4
U = UF * UF                 # 16 filter output channels
TAPS = T * NHB * NHB        # 343
NCORES = 8
HL = H // NCORES            # 16 output rows per core
PIX = HL * W                # 2048 pixels per (b,u) plane
KP = [128, 128, 87]         # tap chunks on the partition axis
KS = [0, 128, 256]
NBU = B * U                 # 32 (b,u) planes

_CACHED = {}


def _build():
    nc = bacc.Bacc("TRN2", target_bir_lowering=False, debug=False,
                   num_devices=NCORES)
    fslab = nc.dram_tensor("fslab", [B, TAPS, U, PIX], F32,
                           kind="ExternalInput")
    ptin = nc.dram_tensor("ptin", [B, C, TAPS, PIX], BF16,
                          kind="ExternalInput")
    nout = nc.dram_tensor("nout", [B, U, C, PIX], F32, kind="ExternalOutput")
    sout = nc.dram_tensor("sout", [NBU * 4, 512], F32, kind="ExternalOutput")

    with tile.TileContext(nc) as tc:
        with tc.tile_pool(name="cst", bufs=1) as cst, \
             tc.tile_pool(name="sb", bufs=2) as sb, \
             tc.tile_pool(name="zp", bufs=2, space="PSUM") as zp, \
             tc.tile_pool(name="sp", bufs=4, space="PSUM") as sp:
            ones = cst.tile([128, 32], BF16)
            nc.vector.memset(ones[:], 1.0)
            zbias = cst.tile([128, 1], F32)
            nc.vector.memset(zbias[:], 0.0)

            # resident patch tiles: 18 x [128, 2048] bf16 = 72KB/partition
            # (loaded lazily: b=0 during bu 0, b=1 just before bu 16 to keep
            # the DMA queue clear for the E-stream pipeline fill)
            pt = {}

            def load_pt(b, c, k):
                kp = KP[k]
                t_ = cst.tile([128, PIX], BF16, name=f"pt{b}{c}{k}")
                nc.scalar.dma_start(t_[:kp, :], ptin[b, c, KS[k]:KS[k] + kp, :])
                pt[b, c, k] = t_

            sps = None  # current S psum tile, 3 slots (partition groups)
            for bu in range(NBU):
                b, u = bu // U, bu % U
                ebf = []
                for k, kp in enumerate(KP):
                    eraw = sb.tile([128, PIX], F32, tag="eraw", bufs=6,
                                   name=f"eraw{bu}_{k}")
                    nc.sync.dma_start(eraw[:kp, :],
                                      fslab[b, KS[k]:KS[k] + kp, u, :])
                    et = sb.tile([128, PIX], BF16, tag="ebf", bufs=6,
                                 name=f"ebf{bu}_{k}")
                    nc.scalar.activation(et[:kp, :], eraw[:kp, :], EXP,
                                         bias=zbias[:kp, :])
                    ebf.append(et)
                if bu == 0:  # first patch loads after bu0's E-stream DMAs
                    for c in range(C):
                        for k in range(len(KP)):
                            load_pt(0, c, k)

                zps = [zp.tile([128, 1024], F32, tag="zps",
                               name=f"zps{bu}_{h}") for h in range(2)]
                for c in range(C):
                    zt = []
                    for k, kp in enumerate(KP):
                        z_ = sb.tile([128, PIX], BF16, tag="z", bufs=6,
                                     name=f"z{bu}_{c}_{k}")
                        nc.vector.tensor_mul(z_[:kp, :], ebf[k][:kp, :],
                                             pt[b, c, k][:kp, :])
                        zt.append(z_)
                    for g in range(4):
                        half, col = g // 2, g % 2
                        out_ap = zps[half][32 * c:32 * c + 32,
                                           512 * col:512 * (col + 1)]
                        for k, kp in enumerate(KP):
                            nc.tensor.matmul(
                                out_ap, ones[:kp, :],
                                zt[k][:kp, 512 * g:512 * (g + 1)],
                                start=(k == 0), stop=(k == 2))

                for half in range(2):
                    zsb = sb.tile([128, 1024], F32, tag="zsb", bufs=6,
                                  name=f"zsb{bu}_{half}")
                    nc.scalar.copy(zsb[:96, :], zps[half][:96, :])
                    nc.scalar.dma_start(
                        nout[b, u, :, 1024 * half:1024 * (half + 1)],
                        zsb[:96:32, :])

                # S stream: sum_p E, 4 col-groups -> slots j=bu*4+g of [128,512]
                for g in range(4):
                    j = bu * 4 + g
                    r = j % 3
                    if r == 0:
                        sps = sp.tile([128, 512], F32, tag="sps",
                                      name=f"sps{j}")
                    for k, kp in enumerate(KP):
                        nc.tensor.matmul(
                            sps[32 * r:32 * r + 32, :], ones[:kp, :],
                            ebf[k][:kp, 512 * g:512 * (g + 1)],
                            start=(k == 0), stop=(k == 2))
                    if r == 2 or j == NBU * 4 - 1:
                        ns = r + 1
                        ssb = sb.tile([128, 512], F32, tag="ssb", bufs=4,
                                      name=f"ssb{j}")
                        nc.scalar.copy(ssb[:32 * ns, :], sps[:32 * ns, :])
                        nc.scalar.dma_start(sout[j - ns + 1:j + 1, :],
                                          ssb[:32 * ns:32, :])
                # b=1 patch loads at body end: 1 tile/bu, behind the
                # current bu's E-stream DMAs in queue order
                if 6 <= bu < 15:
                    i = bu - 6
                    load_pt(1, i // 3, i % 3)
    nc.compile()
    return nc


def _prep_core(x, filt, g):
    """Per-core inputs: filt h-slab + host im2col patch tiles (bf16)."""
    h0 = g * HL
    fslab = np.ascontiguousarray(
        filt[:, :, :, h0:h0 + HL, :]).reshape(B, TAPS, U, PIX)
    xpad = np.pad(x, ((0, 0), (0, 0), (0, 0), (PAD, PAD), (PAD, PAD)))
    win = np.lib.stride_tricks.sliding_window_view(
        xpad[:, :, :, h0:h0 + HL + 2 * PAD, :], (HL, W), axis=(3, 4))
    # win: [B, C, T, 7, 7, HL, W] indexed [b,c,t,i,j,hh,ww]
    ptin = np.ascontiguousarray(win).reshape(B, C, TAPS, PIX)
    return {"fslab": fslab, "ptin": ptin.astype(ml_dtypes.bfloat16)}


def kernel(x: np.ndarray, filt: np.ndarray) -> np.ndarray:
    x = np.asarray(x, dtype=np.float32)
    filt = np.asarray(filt, dtype=np.float32)
    if "nc" not in _CACHED:
        _CACHED["nc"] = _build()
    nc = _CACHED["nc"]

    in_maps = [_prep_core(x, filt, g) for g in range(NCORES)]
    res = run_bass_kernel_spmd(nc, in_maps, list(range(NCORES)))

    out = np.empty((B, C, H * UF, W * UF), np.float32)
    for g in range(NCORES):
        n = res.results[g]["nout"]                       # [B,U,C,PIX]
        s = res.results[g]["sout"].reshape(B, U, PIX)    # [B,U,PIX]
        t = n / s[:, :, None, :]                         # [B,U,C,PIX]
        t = t.reshape(B, UF, UF, C, HL, W)               # [b,r1,r2,c,h,w]
        t = t.transpose(0, 3, 4, 1, 5, 2)                # [b,c,h,r1,w,r2]
        out[:, :, g * HL * UF:(g + 1) * HL * UF, :] = t.reshape(
            B, C, HL * UF, W * UF)
    return out



# revision 6
# speedup vs baseline: 1.0613x; 1.0613x over previous
"""Trainium2 Bass kernel for dynamic-filter 4x upsampling (nn_G_61856118997290).

Math: fw = softmax(filt, axis=1) over 343 taps; per color channel c the
output is pixel-shuffle(sum_p patches(x_c)[p] * fw[p, u]) for u in 0..16.

Computed as E-streams: E = exp(filt - ln16) (scale cancels in N/S),
N_c = sum_p P_c*E, S = sum_p E, out = N_c/S (division on host).

Sharding: output rows H=128 split 8 ways (16 rows/core). Layout: taps padded
343->344 (pad logit -30 => E=0), packed as chunk-pair A [128 parts, 2 ktiles]
(taps j*128+p) plus chunk B [88 parts] (taps 256+p). All device dtypes fp16
(rel err ~1e-3 vs f32 reference).

Per core pipeline:
 - DMA fp16 E-slab tiles; ACT exp(x - ln16) -> E fp16
 - DVE (and gpsimd for a tuned subset) elementwise Z = P*E
 - PE ones-matmuls (M=32 replicated) reduce taps into PSUM partition groups
   {0,32,64,96} = colors 0..2 + S; a tuned subset of bu's computes S on
   gpsimd partition-C reduce instead (host sums the 3 partial rows)
 - ACT evacuates PSUM -> SBUF, DMA to DRAM; host divides N/S + pixel-shuffle.
"""
import math
import numpy as np

import concourse.bass as bass
import concourse.tile as tile
from concourse import bacc, mybir
from concourse.bass_utils import run_bass_kernel_spmd

F32 = mybir.dt.float32
FP16 = mybir.dt.float16
EXP = mybir.ActivationFunctionType.Exp

B, C, T, H, W = 2, 3, 7, 128, 128
NHB, PAD, UF = 7, 3, 4
U = UF * UF                 # 16 filter output channels
TAPS = T * NHB * NHB        # 343
TAPSP = 344                 # padded (tap 343 has logit -30 -> E = 0)
KB = TAPSP - 256            # 88 taps in chunk B
NCORES = 8
HL = H // NCORES            # 16 output rows per core
PIX = HL * W                # 2048 pixels per (b,u) plane
NBU = B * U                 # 32 (b,u) planes
LN16 = float(np.log(16.0))

# --- tuning knobs -----------------------------------------------------------
# bu's whose S-row is reduced on gpsimd (partition-C reduce) instead of PE.
N_POOL_S = 0
# (bu, c) mult pairs executed on gpsimd instead of DVE.
N_POOL_MULT = 0

_CACHED = {}


def _pool_s_set():
    # spread evenly over the 32 bu's
    if N_POOL_S <= 0:
        return set()
    step = NBU / N_POOL_S
    return {int(i * step) for i in range(N_POOL_S)}


def _pool_mult_set():
    if N_POOL_MULT <= 0:
        return set()
    allp = [(bu, c) for bu in range(NBU) for c in range(C)]
    step = len(allp) / N_POOL_MULT
    return {allp[int(i * step)] for i in range(N_POOL_MULT)}


def _build():
    nc = bacc.Bacc("TRN2", target_bir_lowering=False, debug=False,
                   num_devices=NCORES)
    # E-slab packed: A chunk [B, 128, 2, U, PIX] (taps j*128+p), B chunk
    # [B, KB, U, PIX] (taps 256+p)
    fsa = nc.dram_tensor("fsa", [B, 128, 2, U, PIX], FP16, kind="ExternalInput")
    fsb = nc.dram_tensor("fsb", [B, KB, U, PIX], FP16, kind="ExternalInput")
    # patches, same tap packing
    pta = nc.dram_tensor("pta", [B, C, 128, 2, PIX], FP16, kind="ExternalInput")
    ptb = nc.dram_tensor("ptb", [B, C, KB, PIX], FP16, kind="ExternalInput")
    # rows 0..2 = N_c, row 3 = S (PE path)
    outt = nc.dram_tensor("outt", [B, U, 4, PIX], F32, kind="ExternalOutput")
    # 3 partial S rows for pool-S bu's (host sums)
    spart = nc.dram_tensor("spart", [B, U, 3, PIX], F32, kind="ExternalOutput")

    pool_s = _pool_s_set()
    pool_mult = _pool_mult_set()

    with tile.TileContext(nc) as tc:
        with tc.tile_pool(name="cst", bufs=1) as cst, \
             tc.tile_pool(name="sb", bufs=2) as sb, \
             tc.tile_pool(name="zp", bufs=2, space="PSUM") as zp:
            onesA = cst.tile([128, 32], FP16)
            nc.vector.memset(onesA[:], 1.0)
            onesB = cst.tile([KB, 32], FP16)
            nc.vector.memset(onesB[:], 1.0)
            nbias = cst.tile([128, 1], F32)
            nc.vector.memset(nbias[:], -LN16)

            # resident patch tiles
            pa, pb = {}, {}
            for b in range(B):
                for c in range(C):
                    ta = cst.tile([128, 2, PIX], FP16, name=f"pa{b}{c}")
                    nc.sync.dma_start(ta[:], pta[b, c])
                    tb = cst.tile([KB, PIX], FP16, name=f"pb{b}{c}")
                    nc.sync.dma_start(tb[:], ptb[b, c])
                    pa[b, c], pb[b, c] = ta, tb

            for bu in range(NBU):
                b, u = bu // U, bu % U
                ear = sb.tile([128, 2, PIX], FP16, tag="ear", bufs=2,
                              name=f"ear{bu}")
                nc.sync.dma_start(ear[:], fsa[b, :, :, u, :])
                ebr = sb.tile([KB, PIX], FP16, tag="ebr", bufs=2,
                              name=f"ebr{bu}")
                nc.sync.dma_start(ebr[:], fsb[b, :, u, :])
                ea = sb.tile([128, 2, PIX], FP16, tag="ea", bufs=2,
                             name=f"ea{bu}")
                nc.scalar.activation(ea[:], ear[:], EXP, bias=nbias[:])
                eb = sb.tile([KB, PIX], FP16, tag="eb", bufs=2,
                             name=f"eb{bu}")
                nc.scalar.activation(eb[:], ebr[:], EXP, bias=nbias[:KB, :])

                psum = zp.tile([128, 2048], F32, tag="ps", name=f"ps{bu}")
                for c in range(C):
                    za = sb.tile([128, 2, PIX], FP16, tag="za", bufs=3,
                                 name=f"za{bu}{c}")
                    zb = sb.tile([KB, PIX], FP16, tag="zb", bufs=3,
                                 name=f"zb{bu}{c}")
                    eng = nc.gpsimd if (bu, c) in pool_mult else nc.vector
                    eng.tensor_mul(za[:], pa[b, c][:], ea[:])
                    eng.tensor_mul(zb[:], pb[b, c][:], eb[:])
                    for g in range(4):
                        sl = slice(512 * g, 512 * (g + 1))
                        out_ap = psum[32 * c:32 * c + 32, sl]
                        nc.tensor.matmul(out_ap, onesA[:], za[:, 0, sl],
                                         start=True, stop=False)
                        nc.tensor.matmul(out_ap, onesA[:], za[:, 1, sl],
                                         start=False, stop=False)
                        nc.tensor.matmul(out_ap, onesB[:], zb[:, sl],
                                         start=False, stop=True)

                if bu in pool_s:
                    sa = sb.tile([1, 2, PIX], F32, tag="sa", bufs=1,
                                 name=f"sa{bu}")
                    nc.gpsimd.tensor_reduce(sa[:], ea[:], mybir.AxisListType.C,
                                            mybir.AluOpType.add)
                    sbb = sb.tile([1, PIX], F32, tag="sbb", bufs=1,
                                  name=f"sb{bu}")
                    nc.gpsimd.tensor_reduce(sbb[:], eb[:], mybir.AxisListType.C,
                                            mybir.AluOpType.add)
                    nc.sync.dma_start(spart[b, u, 0:2, :], sa[0, :, :])
                    nc.sync.dma_start(spart[b, u, 2, :], sbb[0, :])
                    nprt = 96
                else:
                    for g in range(4):
                        sl = slice(512 * g, 512 * (g + 1))
                        out_ap = psum[96:128, sl]
                        nc.tensor.matmul(out_ap, onesA[:], ea[:, 0, sl],
                                         start=True, stop=False,
                                         tile_position=(0, 96))
                        nc.tensor.matmul(out_ap, onesA[:], ea[:, 1, sl],
                                         start=False, stop=False,
                                         tile_position=(0, 96))
                        nc.tensor.matmul(out_ap, onesB[:], eb[:, sl],
                                         start=False, stop=True,
                                         tile_position=(0, 96))
                    nprt = 128

                zsb = sb.tile([128, 2048], F32, tag="zsb", bufs=2,
                              name=f"zsb{bu}")
                nc.scalar.copy(zsb[:nprt, :], psum[:nprt, :])
                nc.scalar.dma_start(outt[b, u, :nprt // 32, :],
                                    zsb[:nprt:32, :])
    nc.compile()
    return nc


def _prep_core(x, filt, g):
    """Per-core inputs: packed fp16 E-slab + host im2col patch tiles."""
    h0 = g * HL
    slab = np.ascontiguousarray(filt[:, :, :, h0:h0 + HL, :]).reshape(
        B, TAPS, U, PIX)
    slab_p = np.full((B, TAPSP, U, PIX), -30.0, np.float32)
    slab_p[:, :TAPS] = slab
    fsa = slab_p[:, :256].reshape(B, 2, 128, U, PIX).transpose(0, 2, 1, 3, 4)
    fsb = slab_p[:, 256:]

    xpad = np.pad(x, ((0, 0), (0, 0), (0, 0), (PAD, PAD), (PAD, PAD)))
    win = np.lib.stride_tricks.sliding_window_view(
        xpad[:, :, :, h0:h0 + HL + 2 * PAD, :], (HL, W), axis=(3, 4))
    # win: [B, C, T, 7, 7, HL, W] indexed [b,c,t,i,j,hh,ww]
    pt = np.ascontiguousarray(win).reshape(B, C, TAPS, PIX)
    pt_p = np.zeros((B, C, TAPSP, PIX), np.float32)
    pt_p[:, :, :TAPS] = pt
    pta = pt_p[:, :, :256].reshape(B, C, 2, 128, PIX).transpose(0, 1, 3, 2, 4)
    ptb = pt_p[:, :, 256:]
    return {"fsa": np.ascontiguousarray(fsa).astype(np.float16),
            "fsb": np.ascontiguousarray(fsb).astype(np.float16),
            "pta": np.ascontiguousarray(pta).astype(np.float16),
            "ptb": np.ascontiguousarray(ptb).astype(np.float16)}


def kernel(x: np.ndarray, filt: np.ndarray) -> np.ndarray:
    x = np.asarray(x, dtype=np.float32)
    filt = np.asarray(filt, dtype=np.float32)
    if "nc" not in _CACHED:
        _CACHED["nc"] = _build()
    nc = _CACHED["nc"]

    in_maps = [_prep_core(x, filt, g) for g in range(NCORES)]
    res = run_bass_kernel_spmd(nc, in_maps, list(range(NCORES)))

    pool_s = _pool_s_set()
    out = np.empty((B, C, H * UF, W * UF), np.float32)
    for g in range(NCORES):
        o = res.results[g]["outt"]                       # [B,U,4,PIX]
        n = o[:, :, :3]                                  # [B,U,3,PIX]
        s = o[:, :, 3].copy()                            # [B,U,PIX]
        if pool_s:
            sp = res.results[g]["spart"].sum(axis=2)     # [B,U,PIX]
            for bu in pool_s:
                s[bu // U, bu % U] = sp[bu // U, bu % U]
        t = n / s[:, :, None, :]                         # [B,U,C,PIX]
        t = t.reshape(B, UF, UF, C, HL, W)               # [b,r1,r2,c,h,w]
        t = t.transpose(0, 3, 4, 1, 5, 2)                # [b,c,h,r1,w,r2]
        out[:, :, g * HL * UF:(g + 1) * HL * UF, :] = t.reshape(
            B, C, HL * UF, W * UF)
    return out


# revision 45
# speedup vs baseline: 1.2094x; 1.1395x over previous
"""Trainium2 Bass kernel for dynamic-filter 4x upsampling (nn_G_61856118997290).

Math: fw = softmax(filt, axis=1) over 343 taps; per color channel c the
output is pixel-shuffle(sum_p patches(x_c)[p] * fw[p, u]) for u in 0..16.

Computed as E-streams: E = exp(filt - ln16) (scale cancels in N/S),
N_c = sum_p P_c*E, S = sum_p E, out = N_c/S (division on host).

Sharding: output rows H=128 split 8 ways (16 rows/core). Layout: taps padded
343->344 (pad logit -30 => E=0), packed as chunk-pair A [128 parts, 2 ktiles]
(taps j*128+p) plus chunk B [88 parts] (taps 256+p). All device dtypes fp16
(rel err ~1e-3 vs f32 reference).

Per core pipeline:
 - DMA fp16 E-slab tiles; ACT exp(x - ln16) -> E fp16
 - DVE (and gpsimd for a tuned subset) elementwise Z = P*E
 - PE ones-matmuls (M=32 replicated) reduce taps into PSUM partition groups
   {0,32,64,96} = colors 0..2 + S; a tuned subset of bu's computes S on
   gpsimd partition-C reduce instead (host sums the 3 partial rows)
 - ACT evacuates PSUM -> SBUF, DMA to DRAM; host divides N/S + pixel-shuffle.
"""
import math
import numpy as np

import concourse.bass as bass
import concourse.tile as tile
from concourse import bacc, mybir
from concourse.bass_utils import run_bass_kernel_spmd

F32 = mybir.dt.float32
FP16 = mybir.dt.float16
EXP = mybir.ActivationFunctionType.Exp

B, C, T, H, W = 2, 3, 7, 128, 128
NHB, PAD, UF = 7, 3, 4
U = UF * UF                 # 16 filter output channels
TAPS = T * NHB * NHB        # 343
TAPSP = 344                 # padded (tap 343 has logit -30 -> E = 0)
KB = TAPSP - 256            # 88 taps in chunk B
NCORES = 8
HL = H // NCORES            # 16 output rows per core
PIX = HL * W                # 2048 pixels per (b,u) plane
NBU = B * U                 # 32 (b,u) planes
LN16 = float(np.log(16.0))

# --- tuning knobs -----------------------------------------------------------
# bu's whose S-row is reduced on gpsimd (partition-C reduce) instead of PE.
N_POOL_S = 0
# (bu, c) mult pairs executed on gpsimd instead of DVE.
N_POOL_MULT = 0

_CACHED = {}


def _pool_s_set():
    # spread over bu's 4..31 (early bu's stay on PE to avoid pipeline-fill
    # stalls), include the last bu to shorten the PE tail
    if N_POOL_S <= 0:
        return set()
    lo, hi = 4, NBU - 1
    step = (hi - lo) / max(N_POOL_S - 1, 1)
    return {min(hi, lo + int(round(i * step))) for i in range(N_POOL_S)}


def _pool_mult_set():
    if N_POOL_MULT <= 0:
        return set()
    allp = [(bu, c) for bu in range(2, NBU - 2) for c in range(C)]
    step = len(allp) / N_POOL_MULT
    return {allp[min(len(allp) - 1, int(i * step))] for i in range(N_POOL_MULT)}


def _build():
    nc = bacc.Bacc("TRN2", target_bir_lowering=False, debug=False,
                   num_devices=NCORES)
    # E-slab packed: A chunk [B, 128, 2, U, PIX] (taps j*128+p), B chunk
    # [B, KB, U, PIX] (taps 256+p)
    fsa = nc.dram_tensor("fsa", [B, 128, 2, U, PIX], FP16, kind="ExternalInput")
    fsb = nc.dram_tensor("fsb", [B, KB, U, PIX], FP16, kind="ExternalInput")
    # patches, same tap packing
    pta = nc.dram_tensor("pta", [B, C, 128, 2, PIX], FP16, kind="ExternalInput")
    ptb = nc.dram_tensor("ptb", [B, C, KB, PIX], FP16, kind="ExternalInput")
    # rows 0..2 = N_c, row 3 = S (PE path)
    outt = nc.dram_tensor("outt", [B, U, 4, PIX], FP16, kind="ExternalOutput")
    # 3 partial S rows for pool-S bu's (host sums)
    spart = nc.dram_tensor("spart", [B, U, 3, PIX], FP16, kind="ExternalOutput")

    pool_s = _pool_s_set()
    pool_mult = _pool_mult_set()

    with tile.TileContext(nc) as tc:
        with tc.tile_pool(name="cst", bufs=1) as cst, \
             tc.tile_pool(name="sb", bufs=2) as sb, \
             tc.tile_pool(name="zp", bufs=2, space="PSUM") as zp:
            onesA = cst.tile([128, 32], FP16)
            nc.vector.memset(onesA[:], 1.0)
            onesB = cst.tile([KB, 32], FP16)
            nc.vector.memset(onesB[:], 1.0)
            nbias = cst.tile([128, 1], F32)
            nc.vector.memset(nbias[:], -LN16)
            warm = cst.tile([1, 8], FP16)
            nc.vector.memset(warm[:], 0.0)
            nc.scalar.activation(warm[:], warm[:], EXP, bias=nbias[:1, :])

            # patch tiles are resident; E tiles for the first bu's are
            # DMA'd first so the PE pipeline fills quickly, then patches.
            pa, pb = {}, {}
            etiles = {}

            def load_e(bu, split=False):
                b, u = bu // U, bu % U
                ea = sb.tile([128, 2, PIX], FP16, tag="ea", bufs=5,
                             name=f"ea{bu}")
                if split:
                    nc.sync.dma_start(ea[:, 0, :], fsa[b, :, 0, u, :])
                    nc.sync.dma_start(ea[:, 1, :], fsa[b, :, 1, u, :])
                else:
                    nc.sync.dma_start(ea[:], fsa[b, :, :, u, :])
                eb = sb.tile([KB, PIX], FP16, tag="eb", bufs=4,
                             name=f"eb{bu}")
                nc.sync.dma_start(eb[:], fsb[b, :, u, :])
                etiles[bu] = (ea, eb)

            def load_p(b, c):
                ta = cst.tile([128, 2, PIX], FP16, name=f"pa{b}{c}")
                nc.sync.dma_start(ta[:], pta[b, c])
                tb = cst.tile([KB, PIX], FP16, name=f"pb{b}{c}")
                nc.sync.dma_start(tb[:], ptb[b, c])
                pa[b, c], pb[b, c] = ta, tb

            load_e(0, split=True)
            load_p(0, 0)
            load_p(0, 1)
            load_e(1)
            load_p(0, 2)
            load_e(2)
            load_e(3)

            pending = []

            def flush(item):
                fbu, fps0, fps1, fnprt = item
                fb, fu = fbu // U, fbu % U
                zsb = sb.tile([128, 2048], FP16, tag="zsb", bufs=2,
                              name=f"zsb{fbu}")
                nc.scalar.copy(zsb[:fnprt, 0:1024], fps0[:fnprt, :])
                nc.scalar.copy(zsb[:fnprt, 1024:2048], fps1[:fnprt, :])
                nc.scalar.dma_start(outt[fb, fu, :fnprt // 32, :],
                                    zsb[:fnprt:32, :])

            for bu in range(NBU):
                b, u = bu // U, bu % U
                if 4 <= bu + 4 < NBU + 4 and bu + 4 < NBU:
                    load_e(bu + 4)
                if 8 <= bu <= 10:
                    load_p(1, bu - 8)
                ea, eb = etiles.pop(bu)
                # exp in place (frees SBUF for deeper E prefetch); bu0 is
                # split by k-tile so the pipe fills faster
                if bu == 0:
                    nc.scalar.activation(ea[:, 0, :], ea[:, 0, :], EXP,
                                         bias=nbias[:])
                    nc.scalar.activation(ea[:, 1, :], ea[:, 1, :], EXP,
                                         bias=nbias[:])
                else:
                    nc.scalar.activation(ea[:], ea[:], EXP, bias=nbias[:])
                nc.scalar.activation(eb[:], eb[:], EXP, bias=nbias[:KB, :])

                ps0 = zp.tile([128, 1024], F32, tag="ps0", name=f"ps0_{bu}")
                ps1 = zp.tile([128, 1024], F32, tag="ps1", name=f"ps1_{bu}")
                for c in range(C):
                    # pool takes only the small B-chunk mult (short latency)
                    zb_pool = (bu, c) in pool_mult
                    za = sb.tile([128, 2, PIX], FP16, tag="za", bufs=4,
                                 name=f"za{bu}{c}")
                    zbtag = "zbp" if zb_pool else "zb"
                    zb = sb.tile([KB, PIX], FP16, tag=zbtag,
                                 bufs=4 if zb_pool else 4,
                                 name=f"zb{bu}{c}")
                    if bu == 0:
                        nc.vector.tensor_mul(za[:, 0, :], pa[b, c][:, 0, :],
                                             ea[:, 0, :])
                        nc.vector.tensor_mul(za[:, 1, :], pa[b, c][:, 1, :],
                                             ea[:, 1, :])
                    else:
                        nc.vector.tensor_mul(za[:], pa[b, c][:], ea[:])
                    zeng = nc.gpsimd if zb_pool else nc.vector
                    zeng.tensor_mul(zb[:], pb[b, c][:], eb[:])
                    for g in range(4):
                        sl = slice(512 * g, 512 * (g + 1))
                        psel = ps0 if g < 2 else ps1
                        osl = slice(512 * (g % 2), 512 * (g % 2 + 1))
                        out_ap = psel[32 * c:32 * c + 32, osl]
                        nc.tensor.matmul(out_ap, onesA[:], za[:, 0, sl],
                                         start=True, stop=False)
                        nc.tensor.matmul(out_ap, onesA[:], za[:, 1, sl],
                                         start=False, stop=False)
                    for g in range(4):
                        sl = slice(512 * g, 512 * (g + 1))
                        psel = ps0 if g < 2 else ps1
                        osl = slice(512 * (g % 2), 512 * (g % 2 + 1))
                        out_ap = psel[32 * c:32 * c + 32, osl]
                        nc.tensor.matmul(out_ap, onesB[:], zb[:, sl],
                                         start=False, stop=True)


                if bu in pool_s:
                    sa = sb.tile([1, 2, PIX], FP16, tag="sa", bufs=1,
                                 name=f"sa{bu}")
                    sbb = sb.tile([1, PIX], FP16, tag="sbb", bufs=1,
                                  name=f"sb{bu}")
                    with nc.allow_low_precision(reason="S partial rows; host sums in f32"):
                        nc.gpsimd.tensor_reduce(sa[:], ea[:], mybir.AxisListType.C,
                                                mybir.AluOpType.add)
                        nc.gpsimd.tensor_reduce(sbb[:], eb[:], mybir.AxisListType.C,
                                                mybir.AluOpType.add)
                    nc.sync.dma_start(spart[b, u, 0:2, :], sa[0, :, :])
                    nc.sync.dma_start(spart[b, u, 2, :], sbb[0, :])
                    nprt = 96
                else:
                    for g in range(4):
                        sl = slice(512 * g, 512 * (g + 1))
                        psel = ps0 if g < 2 else ps1
                        osl = slice(512 * (g % 2), 512 * (g % 2 + 1))
                        out_ap = psel[96:128, osl]
                        nc.tensor.matmul(out_ap, onesA[:], ea[:, 0, sl],
                                         start=True, stop=False,
                                         tile_position=(0, 96))
                        nc.tensor.matmul(out_ap, onesA[:], ea[:, 1, sl],
                                         start=False, stop=False,
                                         tile_position=(0, 96))
                        nc.tensor.matmul(out_ap, onesB[:], eb[:, sl],
                                         start=False, stop=True,
                                         tile_position=(0, 96))
                    nprt = 128

                pending.append((bu, ps0, ps1, nprt))
                if len(pending) > 1:
                    flush(pending.pop(0))
            for item in pending:
                flush(item)
    nc.compile()
    return nc


def _prep_core(x, filt, g):
    """Per-core inputs: packed fp16 E-slab + host im2col patch tiles."""
    h0 = g * HL
    slab = np.ascontiguousarray(filt[:, :, :, h0:h0 + HL, :]).reshape(
        B, TAPS, U, PIX)
    slab_p = np.full((B, TAPSP, U, PIX), -30.0, np.float32)
    slab_p[:, :TAPS] = slab
    fsa = slab_p[:, :256].reshape(B, 2, 128, U, PIX).transpose(0, 2, 1, 3, 4)
    fsb = slab_p[:, 256:]

    xpad = np.pad(x, ((0, 0), (0, 0), (0, 0), (PAD, PAD), (PAD, PAD)))
    win = np.lib.stride_tricks.sliding_window_view(
        xpad[:, :, :, h0:h0 + HL + 2 * PAD, :], (HL, W), axis=(3, 4))
    # win: [B, C, T, 7, 7, HL, W] indexed [b,c,t,i,j,hh,ww]
    pt = np.ascontiguousarray(win).reshape(B, C, TAPS, PIX)
    pt_p = np.zeros((B, C, TAPSP, PIX), np.float32)
    pt_p[:, :, :TAPS] = pt
    pta = pt_p[:, :, :256].reshape(B, C, 2, 128, PIX).transpose(0, 1, 3, 2, 4)
    ptb = pt_p[:, :, 256:]
    return {"fsa": np.ascontiguousarray(fsa).astype(np.float16),
            "fsb": np.ascontiguousarray(fsb).astype(np.float16),
            "pta": np.ascontiguousarray(pta).astype(np.float16),
            "ptb": np.ascontiguousarray(ptb).astype(np.float16)}


def kernel(x: np.ndarray, filt: np.ndarray) -> np.ndarray:
    x = np.asarray(x, dtype=np.float32)
    filt = np.asarray(filt, dtype=np.float32)
    if "nc" not in _CACHED:
        _CACHED["nc"] = _build()
    nc = _CACHED["nc"]

    in_maps = [_prep_core(x, filt, g) for g in range(NCORES)]
    res = run_bass_kernel_spmd(nc, in_maps, list(range(NCORES)))

    pool_s = _pool_s_set()
    out = np.empty((B, C, H * UF, W * UF), np.float32)
    for g in range(NCORES):
        o = res.results[g]["outt"].astype(np.float32)    # [B,U,4,PIX]
        n = o[:, :, :3]                                  # [B,U,3,PIX]
        s = o[:, :, 3].copy()                            # [B,U,PIX]
        if pool_s:
            sp = res.results[g]["spart"].astype(np.float32).sum(axis=2)
            for bu in pool_s:
                s[bu // U, bu % U] = sp[bu // U, bu % U]
        t = n / s[:, :, None, :]                         # [B,U,C,PIX]
        t = t.reshape(B, UF, UF, C, HL, W)               # [b,r1,r2,c,h,w]
        t = t.transpose(0, 3, 4, 1, 5, 2)                # [b,c,h,r1,w,r2]
        out[:, :, g * HL * UF:(g + 1) * HL * UF, :] = t.reshape(
            B, C, HL * UF, W * UF)
    return out
